# revision 5
# baseline (speedup 1.0000x reference)
"""MoE (8 experts, top-2) Trainium2 kernel.

Strategy (per spec sharding_hint): expert parallelism. The host computes the
(cheap) router — logits, softmax, top-2, renormalized combine weights — and
dispatches each token to the cores owning its two experts ("all-to-all token
dispatch by top-k expert id" done at the sharding step, since kernel() holds
the full inputs host-side). Core e runs the expert-e FFN over its gathered
tokens, capacity-padded so all 8 cores run one SPMD program:

    Y = W2[e]^T @ gelu(W1[e]^T @ XT + b1[e])        (feature-major layouts)

Both weight matrices stay SBUF-resident in bf16 (64 KiB/partition each), so
each 512-token tile runs stage 1 (x@W1 -> GELU -> h in SBUF) immediately
followed by stage 2 (h@W2 -> y): h never leaves SBUF, there is no DRAM
round-trip for the intermediate activation and no phase-boundary PE drain.
All matmuls are bf16 (full PE rate); accumulation is fp32 in PSUM.
The host then scatter-adds  (Y + b2[e]) * combine  back into the output.
"""

import os
import sys

import numpy as np

for _p in ("/opt/trn_rl_repo", "/root/.axon_site/_ro/trn_rl_repo"):
    if os.path.isdir(_p) and _p not in sys.path:
        sys.path.insert(0, _p)

NUM_EXPERTS = 8
TOP_K = 2
B, S, H, I = 4, 4096, 1024, 4096
T = B * S
P = 128
NT = 512           # max token tile = moving free dim (psum bank = 512 fp32)
C_DEFAULT = 4181   # capacity per expert = exact seed-0 max expert count
KH = H // P        # 8 contraction chunks for stage 1
KI = I // P        # 32 contraction chunks for stage 2
OB = H // P        # 8 output blocks for stage 2


def _token_tiles(C):
    """Split C into tiles of 512 plus one arbitrary-size trailing tile."""
    tiles, off = [], 0
    while C - off >= 512:
        tiles.append((off, 512))
        off += 512
    if C - off:
        tiles.append((off, C - off))
        off = C
    return tiles


_built = {}        # C -> nc


def _build(C, reps=1):
    import concourse.bacc as bacc
    import concourse.mybir as mybir
    import concourse.tile as tile
    from concourse._compat import get_trn_type

    f32 = mybir.dt.float32
    bf16 = mybir.dt.bfloat16
    GELU = mybir.ActivationFunctionType.Gelu

    nc = bacc.Bacc(
        get_trn_type() or "TRN2",
        target_bir_lowering=False,
        debug=False,
        enable_asserts=False,
    )
    xt = nc.dram_tensor("xt", [H, C], bf16, kind="ExternalInput").ap()
    w1 = nc.dram_tensor("w1", [H, I], bf16, kind="ExternalInput").ap()
    b1 = nc.dram_tensor("b1", [I], f32, kind="ExternalInput").ap()
    w2 = nc.dram_tensor("w2", [I, H], bf16, kind="ExternalInput").ap()
    y = nc.dram_tensor("y", [H, C], f32, kind="ExternalOutput").ap()

    tiles = _token_tiles(C)
    w1r = w1.rearrange("(ko p) i -> p ko i", p=P)
    w2r = w2.rearrange("(ko p) o -> p ko o", p=P)

    with tile.TileContext(nc) as tc:
        with (
            tc.tile_pool(name="bias", bufs=1) as bpool,
            tc.tile_pool(name="w1p", bufs=1) as w1p,
            tc.tile_pool(name="w2p", bufs=1) as w2p,
            tc.tile_pool(name="xp", bufs=2) as xp,
            tc.tile_pool(name="hp", bufs=1) as hp,
            tc.tile_pool(name="yp", bufs=3) as yp,
            tc.tile_pool(name="ps1p", bufs=4, space="PSUM") as ps1p,
            tc.tile_pool(name="ps2p", bufs=4, space="PSUM") as ps2p,
        ):
            b1sb = bpool.tile([P, KI], f32)
            nc.sync.dma_start(b1sb[:], b1.rearrange("(ib p) -> p ib", p=P))

            for rep in range(reps):
                # Weight loads inside the rep body so the repeated-body
                # timing (bench_reps) charges them, like a fresh call would.
                w1sb = w1p.tile([P, KH, I], bf16, tag="w1",
                                name=f"w1_{rep}")
                w2sb = w2p.tile([P, KI, H], bf16, tag="w2",
                                name=f"w2_{rep}")
                # First w1 column chunk, then x(0), then the rest of w1 in
                # column chunks, then w2 — so PE starts after ~1.3 MB of DMA
                # and each later consumer's data lands before it's needed.
                nc.sync.dma_start(w1sb[:, :, 0:512], w1r[:, :, 0:512])

                xbs = []

                def _load_x(t, toff, tsz, rep=rep):
                    xb = xp.tile([P, KH, tsz], bf16, tag="x",
                                 name=f"x_{rep}_{t}")
                    nc.sync.dma_start(
                        xb[:],
                        xt[:, toff:toff + tsz].rearrange(
                            "(ko p) n -> p ko n", p=P),
                    )
                    return xb

                xbs.append(_load_x(0, tiles[0][0], tiles[0][1]))
                for lo in range(512, I, 512):
                    nc.sync.dma_start(
                        w1sb[:, :, lo:lo + 512], w1r[:, :, lo:lo + 512])
                for c in range(8):
                    cw = KI // 8
                    nc.sync.dma_start(
                        w2sb[:, c * cw:(c + 1) * cw],
                        w2r[:, c * cw:(c + 1) * cw],
                    )

                for t, (toff, tsz) in enumerate(tiles):
                    xb = xbs[0] if t == 0 else _load_x(t, toff, tsz)
                    h = hp.tile([P, KI, tsz], bf16, tag="h",
                                name=f"h_{rep}_{t}")
                    for ib in range(KI):
                        ps = ps1p.tile([P, tsz], f32, tag="ps1",
                                       name=f"ps1_{rep}_{t}_{ib}")
                        for k in range(KH):
                            nc.tensor.matmul(
                                ps[:],
                                lhsT=w1sb[:, k, ib * P:(ib + 1) * P],
                                rhs=xb[:, k],
                                start=(k == 0),
                                stop=(k == KH - 1),
                            )
                        nc.scalar.activation(
                            h[:, ib], ps[:], GELU, bias=b1sb[:, ib:ib + 1]
                        )
                    for ob in range(OB):
                        ps2 = ps2p.tile([P, tsz], f32, tag="ps2",
                                        name=f"ps2_{rep}_{t}_{ob}")
                        for kk in range(KI):
                            nc.tensor.matmul(
                                ps2[:],
                                lhsT=w2sb[:, kk, ob * P:(ob + 1) * P],
                                rhs=h[:, kk],
                                start=(kk == 0),
                                stop=(kk == KI - 1),
                            )
                        ys = yp.tile([P, tsz], f32, tag="y",
                                     name=f"y_{rep}_{t}_{ob}")
                        nc.vector.tensor_copy(ys[:], ps2[:])
                        nc.sync.dma_start(
                            y[ob * P:(ob + 1) * P, toff:toff + tsz], ys[:]
                        )
    nc.finalize()
    return nc


def _routing(hidden, router_w, router_b):
    """Top-2 routing, bit-matching the jax reference on CPU."""
    import jax
    import jax.numpy as jnp

    cpu = jax.local_devices(backend="cpu")[0]
    with jax.default_device(cpu):
        logits = jnp.einsum("bsh,he->bse", jnp.asarray(hidden),
                            jnp.asarray(router_w)) + jnp.asarray(router_b)
        probs = jax.nn.softmax(logits, axis=-1)
        tkp, tki = jax.lax.top_k(probs, TOP_K)
        tkp = tkp / jnp.sum(tkp, axis=-1, keepdims=True)
        tkp_np = np.asarray(tkp).reshape(T, TOP_K)
        tki_np = np.asarray(tki).reshape(T, TOP_K)
    return tkp_np, tki_np


def make_in_maps(x, idx_e, w1, b1, w2, C):
    import ml_dtypes

    bf16 = ml_dtypes.bfloat16
    w1_bf = w1.astype(bf16)
    w2_bf = w2.astype(bf16)
    in_maps = []
    for e in range(NUM_EXPERTS):
        ix = idx_e[e]
        xt = np.zeros((H, C), dtype=bf16)
        xt[:, :len(ix)] = x[ix].astype(bf16).T
        in_maps.append({
            "xt": xt,
            "w1": w1_bf[e],
            "b1": b1[e],
            "w2": w2_bf[e],
        })
    return in_maps


def kernel(hidden_states, w1, b1, w2, b2, router_w, router_b):
    from concourse import bass_utils

    hidden_states = np.ascontiguousarray(hidden_states, dtype=np.float32)
    w1 = np.ascontiguousarray(w1, dtype=np.float32)
    b1 = np.ascontiguousarray(b1, dtype=np.float32)
    w2 = np.ascontiguousarray(w2, dtype=np.float32)
    b2 = np.ascontiguousarray(b2, dtype=np.float32)

    tkp, tki = _routing(hidden_states, router_w, router_b)
    x = hidden_states.reshape(T, H)

    idx_e, prob_e = [], []
    for e in range(NUM_EXPERTS):
        hit = tki == e                       # [T, 2] bool
        idx = np.nonzero(hit.any(axis=1))[0]
        pe = np.where(hit[idx, 0], tkp[idx, 0], tkp[idx, 1]).astype(np.float32)
        idx_e.append(idx)
        prob_e.append(pe)

    maxn = max(len(ix) for ix in idx_e)
    C = C_DEFAULT if maxn <= C_DEFAULT else maxn
    if C not in _built:
        _built[C] = _build(C)
    nc = _built[C]

    in_maps = make_in_maps(x, idx_e, w1, b1, w2, C)

    res = bass_utils.run_bass_kernel_spmd(
        nc, in_maps, core_ids=list(range(NUM_EXPERTS))
    ).results

    out = np.zeros((T, H), dtype=np.float32)
    for e in range(NUM_EXPERTS):
        ix = idx_e[e]
        yv = res[e]["y"][:, :len(ix)].T
        out[ix] += (yv + b2[e]) * prob_e[e][:, None]
    return out.reshape(B, S, H)
